# revision 1
# baseline (speedup 1.0000x reference)
"""Causal MHA (B=4, S=2048, D=1024, H=16) on 8 TRN2 cores, head-parallel.

Core c = (batch b=c//2, head-half hh=c%2): computes Q/K/V for its 8 heads
over all 2048 tokens (no duplication), full causal attention for those heads,
and a partial output projection out_part = (ctx/den) @ Wo[:, half].T.
Host sums the two partials per batch and adds bo + Wo@bv (the V bias passes
through softmax unchanged).

Every core runs the identical causal triangle -> one SPMD graph, zero
collectives, zero union-padding waste. Attention uses transposed scores
[k, q]: per head pair and k-tile, two 512-col score matmuls fill adjacent
PSUM banks, one fused ScalarE exp (scale=1/8, bias=key-padding) writes bf16
E; ctx accumulates per pair (2 heads col-packed); softmax denominator
accumulates on DVE in bf16 with a final ones-matmul broadcast feeding the
reciprocal normalize.
"""

import os
import sys

sys.path.insert(0, "/opt/trn_rl_repo")

import numpy as np
import ml_dtypes

import concourse.bass as bass
import concourse.bacc as bacc
import concourse.tile as tile
from concourse import mybir
from concourse.bass_utils import run_bass_kernel_spmd

B, S, D, H = 4, 2048, 1024, 16
HD = D // H  # 64
P = 128
KC = D // P   # 8 contraction chunks for QKV projections
KC2 = 4       # contraction chunks for O projection (512 dims)
QW = 512      # query stripe width
NS = S // QW  # 4 stripes
NEG = -1e30
BF16 = mybir.dt.bfloat16
F32 = mybir.dt.float32
NPBF16 = ml_dtypes.bfloat16


def _build():
    nc = bacc.Bacc()

    xt = nc.declare_dram_parameter("xt", [P, KC, S], BF16, isOutput=False)
    wq = nc.declare_dram_parameter("wq", [P, KC, 512], BF16, isOutput=False)
    wk = nc.declare_dram_parameter("wk", [P, KC, 512], BF16, isOutput=False)
    wv = nc.declare_dram_parameter("wv", [P, KC, 512], BF16, isOutput=False)
    wo = nc.declare_dram_parameter("wo", [P, KC2, D], BF16, isOutput=False)
    bqp = nc.declare_dram_parameter("bqp", [P, KC2], F32, isOutput=False)
    bkp = nc.declare_dram_parameter("bkp", [P, KC2], F32, isOutput=False)
    pad = nc.declare_dram_parameter("pad", [P, S // P], F32, isOutput=False)
    tri = nc.declare_dram_parameter("tri", [P, QW], BF16, isOutput=False)
    eye = nc.declare_dram_parameter("eye", [P, P], BF16, isOutput=False)
    out = nc.declare_dram_parameter("out", [S, D], BF16, isOutput=True)

    from contextlib import ExitStack

    with tile.TileContext(nc) as tc, ExitStack() as ctx:
        wpool = ctx.enter_context(tc.tile_pool(name="wpool", bufs=1))
        xpool = ctx.enter_context(tc.tile_pool(name="xpool", bufs=2))
        bigpool = ctx.enter_context(tc.tile_pool(name="bigpool", bufs=1))
        epool = ctx.enter_context(tc.tile_pool(name="epool", bufs=35))
        spool = ctx.enter_context(tc.tile_pool(name="spool", bufs=6))
        pp_acc = ctx.enter_context(tc.tile_pool(name="pp_acc", bufs=2, space="PSUM"))
        pp_sc = ctx.enter_context(tc.tile_pool(name="pp_sc", bufs=2, space="PSUM"))
        pp_ctx = ctx.enter_context(tc.tile_pool(name="pp_ctx", bufs=1, space="PSUM"))

        # ---- constants into SBUF ----
        wq_s = wpool.tile([P, KC, 512], BF16, tag="wq")
        wk_s = wpool.tile([P, KC, 512], BF16, tag="wk")
        wv_s = wpool.tile([P, KC, 512], BF16, tag="wv")
        wo_s = wpool.tile([P, KC2, D], BF16, tag="wo")
        bq_s = wpool.tile([P, KC2], F32, tag="bq")
        bk_s = wpool.tile([P, KC2], F32, tag="bk")
        pad_s = wpool.tile([P, S // P], F32, tag="pad")
        tri_s = wpool.tile([P, QW], BF16, tag="tri")
        eye_s = wpool.tile([P, P], BF16, tag="eye")
        # touch Exp once at t=0 so the ~1.3us ACT table load happens inside
        # the startup DMA shadow, not at the first real softmax
        warm_s = wpool.tile([P, 1], F32, tag="warm")
        nc.vector.memset(warm_s[:], 0.0)
        nc.scalar.activation(warm_s[:], warm_s[:],
                             mybir.ActivationFunctionType.Exp, scale=1.0)

        # ---- big persistent activations ----
        qT_s = bigpool.tile([P, KC2, S], BF16, tag="qT")   # [pairdims, pair, q]
        kT_s = bigpool.tile([P, KC2, S], BF16, tag="kT")   # [pairdims, pair, k]
        v_s = bigpool.tile([P, S // P, 8, HD + 1], BF16, tag="v")  # [k, ktile, h, hd|1]
        nc.vector.memset(v_s[:, :, :, HD:HD + 1], 1.0)
        cT_s = bigpool.tile([P, KC2, S], BF16, tag="cT")   # [pairdims, pair, q]

        def load_xt(st):
            ssl = slice(st * QW, (st + 1) * QW)
            xt_t = xpool.tile([P, KC, QW], BF16, tag="xt")
            nc.sync.dma_start(xt_t[:], xt[:, :, ssl])
            return xt_t

        def qkv_tiles(st, xt_t, parts="qkv", on_act=False):
            """Thunks projecting tokens [st*512, (st+1)*512): Q stripe st,
            K/V k-tiles 4*st..4*st+3. Each psum tile is split into two
            ~850ns emission units so it can interleave at j granularity.
            on_act routes the PSUM->SBUF writes to ScalarE (Identity+bias)
            for stages whose filler window has a saturated DVE."""
            ssl = slice(st * QW, (st + 1) * QW)
            thunks = []
            state = {}

            def proj_half(w_s, b_s, dst, m, half, key):
                if half == 0:
                    state[key] = pp_acc.tile([P, QW], F32, tag="acc", name="acc_ps")
                ps = state[key]
                for kc in range(4 * half, 4 * half + 4):
                    nc.tensor.matmul(
                        ps[:], lhsT=w_s[:, kc, m * P:(m + 1) * P],
                        rhs=xt_t[:, kc, :],
                        start=(kc == 0), stop=(kc == KC - 1))
                if half == 1:
                    if on_act:
                        nc.scalar.activation(dst[:, m, ssl], ps[:],
                                             mybir.ActivationFunctionType.Identity,
                                             bias=b_s[:, m:m + 1], scale=1.0)
                    else:
                        nc.vector.tensor_scalar_add(dst[:, m, ssl], ps[:],
                                                    b_s[:, m:m + 1])
                    del state[key]

            def v_half(sub, half, key):
                if half == 0:
                    state[key] = pp_acc.tile([P, 8, HD], F32, tag="acc",
                                             name="acc_ps")
                ps = state[key]
                for kc in range(4 * half, 4 * half + 4):
                    nc.tensor.matmul(
                        ps[:], lhsT=xt_t[:, kc, sub * P:(sub + 1) * P],
                        rhs=wv_s[:, kc, :],
                        start=(kc == 0), stop=(kc == KC - 1))
                if half == 1:
                    if on_act:
                        nc.scalar.activation(v_s[:, st * 4 + sub, :, 0:HD],
                                             ps[:],
                                             mybir.ActivationFunctionType.Copy,
                                             scale=1.0)
                    else:
                        nc.vector.tensor_copy(out=v_s[:, st * 4 + sub, :, 0:HD],
                                              in_=ps[:])
                    del state[key]

            plan = []
            if "q" in parts:
                plan.append((wq_s, bq_s, qT_s))
            if "k" in parts:
                plan.append((wk_s, bk_s, kT_s))
            for pi, (w_s, b_s, dst) in enumerate(plan):
                for m in range(KC2):
                    for half in range(2):
                        thunks.append(
                            (852, lambda w_s=w_s, b_s=b_s, dst=dst, m=m,
                             half=half, key=(pi, m):
                             proj_half(w_s, b_s, dst, m, half, key)))
            if "v" in parts:
                for sub in range(4):
                    for half in range(2):
                        thunks.append((852, lambda sub=sub, half=half,
                                       key=("v", sub): v_half(sub, half, key)))
            return thunks

        def qkv_stage(st, xt_t, parts="qkv"):
            for _, t in qkv_tiles(st, xt_t, parts):
                t()

        from collections import deque
        fillers = deque()

        def drain():
            while fillers:
                fillers.popleft()[1]()

        attn_state = {}

        def attn_ph1(stripe, pr, j):
            """Scores/exp/mask for one (pair, k-tile); e kept in SBUF."""
            es = attn_state[(stripe, pr)]
            m = j
            ksl = slice(m * P, (m + 1) * P)
            diag = m >= 4 * stripe
            # within a diagonal k-tile of shift t = m-4s, queries below
            # t*128 are entirely masked -- compute only the valid sub-range
            off = (m - 4 * stripe) * P if diag else 0
            w = QW - off
            qsub = slice(stripe * QW + off, (stripe + 1) * QW)
            # scores for both heads of the pair in adjacent banks
            sc = pp_sc.tile([P, 2 * QW], F32, tag="sc")
            for q_i in range(2):
                lo = q_i * HD
                nc.tensor.matmul(
                    sc[:, q_i * QW + off:(q_i + 1) * QW],
                    lhsT=kT_s[lo:lo + HD, pr, ksl],
                    rhs=qT_s[lo:lo + HD, pr, qsub],
                    start=True, stop=True, tile_position=(lo, 0))
            e = epool.tile([P, 2 * QW], BF16, tag="e")
            es.append(e)
            if stripe >= 2 and off >= 256:
                # stripe 3 is Act-bound: narrow diagonal exps pay for their
                # second instruction there
                for q_i in range(2):
                    esl = slice(q_i * QW + off, (q_i + 1) * QW)
                    nc.scalar.activation(e[:, esl], sc[:, esl],
                                         mybir.ActivationFunctionType.Exp,
                                         bias=pad_s[:, m:m + 1],
                                         scale=0.125)
            else:
                # full-width exp: any masked query columns hold garbage
                # (stale PSUM) but are never read downstream
                nc.scalar.activation(e[:], sc[:],
                                     mybir.ActivationFunctionType.Exp,
                                     bias=pad_s[:, m:m + 1],
                                     scale=0.125)
            if diag:
                # only the 128-wide diagonal query block needs the triangle;
                # beyond it tri is all-ones (no-op)
                for q_i in range(2):
                    esl = slice(q_i * QW + off, q_i * QW + off + P)
                    nc.vector.tensor_tensor(
                        e[:, esl], e[:, esl], tri_s[:, 0:P],
                        mybir.AluOpType.mult)

        def attn_ph2_units(stripe, pr):
            """Post-phase-1 units for a pair, each an atomic thunk: 8
            consecutive [q, hd|den] ctx accumulation bursts (65-col matmuls
            whose ones-column of V accumulates the softmax denominator for
            exactly the causal k-range of each query block), the strided
            reciprocal, 8 fused normalize-copies to SBUF, 8 identity
            transposes back to [hhd, q], and the PSUM->cT copy."""
            qsl = slice(stripe * QW, (stripe + 1) * QW)
            es = attn_state[(stripe, pr)]
            nkt = 4 * stripe + 4
            st = {}

            def region(q_i, qb):
                if (q_i, qb) == (0, 0):
                    st["ctx_ps"] = pp_ctx.tile([P, 2, QW], F32, tag="ctx",
                                               name="ctx_ps")
                ctx_ps = st["ctx_ps"]
                h = 2 * pr + q_i
                js = list(range(0, 4 * stripe + qb + 1))
                for i, j in enumerate(js):
                    nc.tensor.matmul(
                        ctx_ps[:, q_i, qb * (HD + 1):(qb + 1) * (HD + 1)],
                        lhsT=es[j][:, q_i * QW + qb * P:q_i * QW + (qb + 1) * P],
                        rhs=v_s[:, j, h, :],
                        start=(i == 0), stop=(i == len(js) - 1),
                        skip_group_check=True)

            def recip():
                st["rden"] = spool.tile([P, 2, 4], F32, tag="rden",
                                        name="rden", bufs=2)
                nc.vector.reciprocal(
                    st["rden"][:],
                    st["ctx_ps"][:, :, HD:4 * (HD + 1):HD + 1])

            def copyraw():
                # one bf16 copy frees the 2-bank ctx slot immediately; the
                # per-region normalizes then read SBUF at 2x DVE rate
                st["craw"] = spool.tile([P, 2, QW], BF16, tag="craw",
                                        name="craw", bufs=2)
                nc.vector.tensor_copy(out=st["craw"][:], in_=st["ctx_ps"][:])
                del st["ctx_ps"]
                attn_state[(stripe, pr)] = []  # release e tiles

            def norm(q_i, qb):
                nc.vector.tensor_scalar_mul(
                    st["ctxc"][:, (q_i * 4 + qb) * HD:(q_i * 4 + qb + 1) * HD],
                    st["craw"][:, q_i, qb * (HD + 1):qb * (HD + 1) + HD],
                    st["rden"][:, q_i, qb:qb + 1])

            def mkctxc():
                st["ctxc"] = spool.tile([P, QW], BF16, tag="ctxc",
                                        name="ctxc", bufs=2)

            def transpose():
                st["pt"] = pp_ctx.tile([P, 2, QW], F32, tag="ctx", name="pt")
                for q_i in range(2):
                    for qb in range(4):
                        nc.tensor.matmul(
                            st["pt"][q_i * HD:(q_i + 1) * HD, 0,
                                     qb * P:(qb + 1) * P],
                            lhsT=st["ctxc"][:, (q_i * 4 + qb) * HD:
                                            (q_i * 4 + qb + 1) * HD],
                            rhs=eye_s[:],
                            start=True, stop=True,
                            skip_group_check=True)

            def ptcopy():
                nc.vector.tensor_copy(out=cT_s[:, pr, qsl],
                                      in_=st["pt"][:, 0, :])
                del attn_state[(stripe, pr)]

            units = [(27 * (4 * stripe + qb + 1),
                      lambda q_i=q_i, qb=qb: region(q_i, qb))
                     for q_i in range(2) for qb in range(4)]
            units += [(0, recip), (0, copyraw), (0, mkctxc)]
            units += [(0, lambda q_i=q_i, qb=qb: norm(q_i, qb))
                      for q_i in range(2) for qb in range(4)]
            units += [(427, transpose), (0, ptcopy)]
            return units

        pending = deque()  # ph2 units of the previously finished pair
        carry = [0]        # un-met PE deficit banked across k-tiles

        def attn_pair(stripe, pr, budget_ns=350):
            """Phase 1 j-loop for one pair, interleaving ph2 units of the
            previous pair (and fillers) between k-tiles. The exp costs
            ~1038ns/k-tile vs ~426ns of scores, so ~612ns of PE filler per
            k-tile keeps the PE dense; the unmet remainder banks forward."""
            nkt = 4 * stripe + 4
            attn_state[(stripe, pr)] = []
            for j in range(nkt):
                attn_ph1(stripe, pr, j)
                budget = carry[0] + budget_ns
                while budget > 100:
                    if pending:
                        ns, t = pending.popleft()
                    elif fillers:
                        ns, t = fillers.popleft()
                    else:
                        break
                    t()
                    budget -= max(ns, 50)
                carry[0] = min(max(budget, 0), 1500)
            pending.extend(attn_ph2_units(stripe, pr))

        def oproj_tiles(t8, alt=False, quarters=False):
            """Thunks for one 128-token output block: per 512-dout half,
            either one ~850ns thunk or two ~430ns halves (quarters=True)."""
            osl = slice(t8 * P, (t8 + 1) * P)
            state = {}

            def mms(dt, kcs, ps):
                dsl = slice(dt * QW, (dt + 1) * QW)
                for kc in kcs:
                    nc.tensor.matmul(ps[:], lhsT=cT_s[:, kc, osl],
                                     rhs=wo_s[:, kc, dsl],
                                     start=(kc == 0), stop=(kc == KC2 - 1))

            def fin(dt, ps):
                # both 512-halves stage into one tile; a single [128, 1024]
                # DMA per token block halves HWDGE/semaphore traffic
                if dt == 0:
                    state["ob"] = spool.tile([P, 2, QW], BF16, tag="outsb",
                                             name="ob")
                ob = state["ob"]
                if alt and dt == 1:
                    nc.scalar.activation(ob[:, dt, :], ps[:],
                                         mybir.ActivationFunctionType.Copy,
                                         scale=1.0)
                else:
                    nc.vector.tensor_copy(out=ob[:, dt, :], in_=ps[:])
                if dt == 1:
                    (nc.scalar if alt else nc.sync).dma_start(
                        out[osl, :], ob[:, :, :].opt() if False else ob[:])
                    del state["ob"]

            def whole(dt):
                if alt and dt == 1:
                    ps = pp_sc.tile([P, 2 * QW], F32, tag="sc",
                                    name="oproj_ps")[:, 0:QW]
                else:
                    ps = pp_acc.tile([P, QW], F32, tag="acc", name="oproj_ps")
                mms(dt, range(KC2), ps)
                fin(dt, ps)

            def qopen(dt):
                state[dt] = pp_acc.tile([P, QW], F32, tag="acc",
                                        name="oproj_ps")
                mms(dt, range(2), state[dt])

            def qclose(dt):
                ps = state.pop(dt)
                mms(dt, range(2, KC2), ps)
                fin(dt, ps)

            if quarters:
                return [lambda dt=dt, f=f: f(dt)
                        for dt in range(2) for f in (qopen, qclose)]
            return [lambda dt=dt: whole(dt) for dt in range(2)]

        def oproj(t8, alt=False):
            for t in oproj_tiles(t8, alt):
                t()

        held = {}

        def oproj_open(t8, dt):
            """kc 0..2 of a final oproj tile -- legal once pairs 0..2 of
            stripe 3 are normalized; only kc=3 waits on the last pair."""
            ps = pp_acc.tile([P, QW], F32, tag="acc", name="oproj_ps")
            dsl = slice(dt * QW, (dt + 1) * QW)
            osl = slice(t8 * P, (t8 + 1) * P)
            for kc in range(3):
                nc.tensor.matmul(ps[:], lhsT=cT_s[:, kc, osl],
                                 rhs=wo_s[:, kc, dsl],
                                 start=(kc == 0), stop=False)
            held[(t8, dt)] = ps

        heldob = {}

        def oproj_close(t8, dt):
            ps = held.pop((t8, dt))
            dsl = slice(dt * QW, (dt + 1) * QW)
            osl = slice(t8 * P, (t8 + 1) * P)
            nc.tensor.matmul(ps[:], lhsT=cT_s[:, 3, osl],
                             rhs=wo_s[:, 3, dsl],
                             start=False, stop=True)
            if dt == 0:
                heldob[t8] = spool.tile([P, 2, QW], BF16, tag="outsb",
                                        name="ob")
            ob = heldob[t8]
            nc.vector.tensor_copy(out=ob[:, dt, :], in_=ps[:])
            if dt == 1:
                nc.sync.dma_start(out[osl, :], ob[:])
                del heldob[t8]

        # ---- schedule ----
        # Startup DMAs split across both HWDGE queues (SP + Activation) and
        # chunked so the first Q-projection matmuls can start ~3us in.
        xt0 = xpool.tile([P, KC, QW], BF16, tag="xt")
        nc.scalar.dma_start(xt0[:, 0:2, :], xt[:, 0:2, 0:QW])
        nc.sync.dma_start(wq_s[:, 0:2, 0:128], wq[:, 0:2, 0:128])
        nc.scalar.dma_start(xt0[:, 2:4, :], xt[:, 2:4, 0:QW])
        nc.sync.dma_start(wq_s[:, 2:4, 0:128], wq[:, 2:4, 0:128])
        nc.scalar.dma_start(xt0[:, 4:6, :], xt[:, 4:6, 0:QW])
        nc.sync.dma_start(wq_s[:, 4:8, 0:128], wq[:, 4:8, 0:128])
        nc.scalar.dma_start(xt0[:, 6:8, :], xt[:, 6:8, 0:QW])
        nc.sync.dma_start(wq_s[:, :, 128:256], wq[:, :, 128:256])
        nc.sync.dma_start(wq_s[:, :, 256:512], wq[:, :, 256:512])
        nc.sync.dma_start(bq_s[:], bqp[:])
        nc.scalar.dma_start(bk_s[:], bkp[:])
        nc.sync.dma_start(wk_s[:, :, 0:256], wk[:, :, 0:256])
        nc.scalar.dma_start(wk_s[:, :, 256:512], wk[:, :, 256:512])
        nc.sync.dma_start(pad_s[:], pad[:])
        nc.scalar.dma_start(tri_s[:], tri[:])
        nc.sync.dma_start(wv_s[:, 4:8, :], wv[:, 4:8, :])
        nc.scalar.dma_start(wv_s[:, 0:4, :], wv[:, 0:4, :])
        nc.scalar.dma_start(eye_s[:], eye[:])
        qkv_stage(0, xt0, parts="q")
        qkv_stage(0, xt0, parts="k")
        qkv_stage(0, xt0, parts="v")
        xt1 = load_xt(1)
        nc.sync.dma_start(wo_s[:], wo[:])
        fillers.extend(qkv_tiles(1, xt1, parts="qk"))
        fillers.extend(qkv_tiles(1, xt1, parts="v", on_act=True))
        for pr in range(4):
            attn_pair(0, pr)
        drain()
        xt2 = load_xt(2)
        fillers.extend(qkv_tiles(2, xt2, parts="qk"))
        fillers.extend(qkv_tiles(2, xt2, parts="v", on_act=True))
        for pr in range(4):
            attn_pair(1, pr)
        drain()
        xt3 = load_xt(3)
        fillers.extend(qkv_tiles(3, xt3, parts="qk"))
        for pr in range(4):
            attn_pair(2, pr)
        drain()
        # stripe 3 is Act-bound: stuff every movable PE item into it.
        # V(3) is safe here -- first needed by the ctx regions of pair (3,0),
        # which only run during ph1(3,1).
        fillers.extend(qkv_tiles(3, xt3, parts="v"))
        for t8 in range(0, 11):                          # 44 quarter-thunks
            fillers.extend((426, t) for t in oproj_tiles(t8, quarters=True))
        for pr in range(3):
            attn_pair(3, pr, budget_ns=450)
        # extra late-legal PE work for the last pair's ph1 (popped strictly
        # after pair (3,2)'s pending units, so after its normalize):
        # oproj 11 + the kc 0..2 partials of t8 12
        fillers.extend((426, t) for t in oproj_tiles(11, quarters=True))
        fillers.append((640, lambda: oproj_open(12, 0)))
        fillers.append((640, lambda: oproj_open(12, 1)))
        attn_pair(3, 3, budget_ns=450)
        while pending:
            pending.popleft()[1]()
            if fillers:
                fillers.popleft()[1]()
        drain()
        oproj_close(12, 0)
        oproj_close(12, 1)
        for t8 in range(13, 16):
            oproj(t8, alt=True)

    nc.compile()
    return nc


def _core_inputs(c, x, padding_mask, Wq, bq, Wk, bk, Wv, bv, Wo, bo):
    b, hh = c // 2, c % 2
    hsl = slice(hh * 512, (hh + 1) * 512)

    xt = np.ascontiguousarray(
        x[b].T.reshape(KC, P, S).transpose(1, 0, 2)).astype(NPBF16)

    def wl(Wh):  # [512 out, 1024 in] -> [P, KC, 512]
        return np.ascontiguousarray(
            Wh.T.reshape(KC, P, 512).transpose(1, 0, 2)).astype(NPBF16)

    wol = np.ascontiguousarray(
        Wo[:, hsl].T.reshape(KC2, P, D).transpose(1, 0, 2)).astype(NPBF16)

    bqp = np.ascontiguousarray(bq[hsl].reshape(KC2, P).T).astype(np.float32)
    bkp = np.ascontiguousarray(bk[hsl].reshape(KC2, P).T).astype(np.float32)

    padb = np.where(padding_mask[b].reshape(S // P, P).T, 0.0,
                    NEG).astype(np.float32)
    padb = np.ascontiguousarray(padb)

    kk = np.arange(P)[:, None]
    uu = np.arange(QW)[None, :]
    trib = np.ascontiguousarray((kk <= uu).astype(NPBF16))

    return {"xt": xt, "wq": wl(Wq[hsl]), "wk": wl(Wk[hsl]), "wv": wl(Wv[hsl]),
            "wo": wol, "bqp": bqp, "bkp": bkp, "pad": padb, "tri": trib,
            "eye": np.eye(P, dtype=NPBF16)}


_NC_CACHE = {}


def kernel(x, padding_mask, Wq, bq, Wk, bk, Wv, bv, Wo, bo):
    x = np.asarray(x, np.float32)
    padding_mask = np.asarray(padding_mask, bool)
    args = [np.asarray(a, np.float32) for a in (Wq, bq, Wk, bk, Wv, bv, Wo, bo)]

    if "nc" not in _NC_CACHE:
        _NC_CACHE["nc"] = _build()
    nc = _NC_CACHE["nc"]

    in_maps = [_core_inputs(c, x, padding_mask, *args) for c in range(8)]

    trace = bool(int(os.environ.get("KERNEL_TRACE", "0")))
    try:
        res = run_bass_kernel_spmd(nc, in_maps, core_ids=list(range(8)), trace=trace)
    except ModuleNotFoundError:
        res = run_bass_kernel_spmd(nc, in_maps, core_ids=list(range(8)))
    if trace and res.exec_time_ns is not None:
        print(f"HW exec time: {res.exec_time_ns} ns")
        _NC_CACHE["exec_time_ns"] = res.exec_time_ns

    Wo_, bv_, bo_ = args[6], args[5], args[7]
    btot = (bo_ + Wo_ @ bv_).astype(np.float32)
    full = np.empty((B, S, D), np.float32)
    for b in range(B):
        full[b] = (res.results[2 * b]["out"].astype(np.float32)
                   + res.results[2 * b + 1]["out"].astype(np.float32) + btot)
    return full


if __name__ == "__main__":
    rng = np.random.default_rng(0)
    x = rng.standard_normal((B, S, D), dtype=np.float32)
    lengths = rng.integers(S // 2, S + 1, size=(B,))
    pm = np.arange(S)[None, :] < lengths[:, None]
    std = 0.02
    ws = {n: (rng.standard_normal((D, D), dtype=np.float32) * std)
          for n in ("Wq", "Wk", "Wv", "Wo")}
    z = np.zeros((D,), np.float32)
    out = kernel(x, pm, ws["Wq"], z, ws["Wk"], z, ws["Wv"], z, ws["Wo"], z)
    print(out.shape, out.dtype, np.abs(out).mean())

